# revision 1
# baseline (speedup 1.0000x reference)
"""Trainium2 Bass kernel for the hard-negative-mining set loss (v4).

Structure:
  * phase A: per-class hardest-negative mining in exp domain
    (negq = -exp(x-SH)/rsum - onehot), PE-transpose to [class,row],
    DVE top-8 + first-index per class
  * phase B: per-class first/second occurrence (pos candidates)
  * one 16KB AllGather; every core resolves global per-class tables
  * per-anchor indices via one-hot matmul gather; paired indirect-DMA row
    gathers; CE with constant-shift softmax; Lns dep-pinned after Exps.
"""

import numpy as np

import concourse.bass as bass
import concourse.bacc as bacc
import concourse.tile as tile
from concourse import mybir
from concourse.bass_utils import run_bass_kernel_spmd
from concourse.tile import add_dep_helper

B, C = 8192, 1024
NCORES = 8
BL = B // NCORES      # 1024 local rows per core
NT = BL // 128        # 8 row tiles
CT = C // 128         # 8 class tiles
BIGI = 16384.0        # index encoding base: enc = BIGI - global_row_idx
SHIFT_A = 10.0        # softmax shift (x ~ N(0,1): rowmax << SHIFT_A)
SHIFT_C = 14.0        # summed-logits shift (3 logits per entry)
F32 = mybir.dt.float32
I32 = mybir.dt.int32
U32 = mybir.dt.uint32
AX = mybir.AxisListType
OP = mybir.AluOpType
AF = mybir.ActivationFunctionType


def build_nc():
    nc = bacc.Bacc("TRN2", target_bir_lowering=False, debug=False,
                   num_devices=NCORES)

    x_d = nc.dram_tensor("x", [B, C], F32, kind="ExternalInput")
    xloc_d = nc.dram_tensor("xloc", [BL, C], F32, kind="ExternalInput")
    cidb_d = nc.dram_tensor("cidb", [128, C], F32, kind="ExternalInput")
    tgtb_d = nc.dram_tensor("tgtb", [128, BL], F32, kind="ExternalInput")
    negjb_d = nc.dram_tensor("negjb", [128, BL], F32, kind="ExternalInput")
    ident_d = nc.dram_tensor("ident", [128, 128], F32, kind="ExternalInput")
    tcols_d = nc.dram_tensor("tcols", [128, NT], F32, kind="ExternalInput")
    gidxcol_d = nc.dram_tensor("gidxcol", [128, NT], F32, kind="ExternalInput")
    cidcol_d = nc.dram_tensor("cidcol", [128, CT], F32, kind="ExternalInput")
    bigoff_d = nc.dram_tensor("bigoff", [128, 1], F32, kind="ExternalInput")
    out_d = nc.dram_tensor("partial", [1, 1], F32, kind="ExternalOutput")

    # collective bounce, partition-major; col q*CT+ct = class ct*128+p,
    # quantity q: 0=vmax 1=negenc 2=f1enc 3=f2enc
    cc_in = nc.dram_tensor("cc_in", [128, 4 * CT], F32)
    cc_out = nc.dram_tensor("cc_out", [NCORES, 128, 4 * CT], F32)

    with tile.TileContext(nc) as tc:
        with (
            tc.tile_pool(name="persist", bufs=1) as pp,
            tc.tile_pool(name="scratch", bufs=2) as sp,
            tc.tile_pool(name="dumppool", bufs=3) as dp,
            tc.tile_pool(name="nvpool", bufs=8) as nvp,
            tc.tile_pool(name="small", bufs=4) as smp,
            tc.tile_pool(name="gather", bufs=2) as gp,
            tc.tile_pool(name="psA", bufs=1, space="PSUM") as psa,
            tc.tile_pool(name="psB", bufs=2, space="PSUM") as psb,
        ):
            # ---------- input loads (xloc first: gates phase A) ----------
            xloc = []
            for t in range(NT):
                xt = pp.tile([128, C], F32, tag=f"xloc{t}")
                nc.sync.dma_start(out=xt, in_=xloc_d.ap()[t * 128:(t + 1) * 128, :])
                xloc.append(xt)

            tcols = pp.tile([128, NT], F32, tag="tcols")
            nc.sync.dma_start(out=tcols, in_=tcols_d.ap())
            gidxcol = pp.tile([128, NT], F32, tag="gidxcol")
            nc.sync.dma_start(out=gidxcol, in_=gidxcol_d.ap())
            cidcol = pp.tile([128, CT], F32, tag="cidcol")
            nc.sync.dma_start(out=cidcol, in_=cidcol_d.ap())
            bigoff = pp.tile([128, 1], F32, tag="bigoff")
            nc.sync.dma_start(out=bigoff, in_=bigoff_d.ap())

            ident = pp.tile([128, 128], F32, tag="ident")
            nc.sync.dma_start(out=ident, in_=ident_d.ap())
            ones = pp.tile([128, 1], F32, tag="ones")
            nc.vector.memset(ones, 1.0)
            shA = pp.tile([128, 1], F32, tag="shA")
            nc.vector.memset(shA, -SHIFT_A)
            shC = pp.tile([128, 1], F32, tag="shC")
            nc.vector.memset(shC, -SHIFT_C)

            cidb = pp.tile([128, C], F32, tag="cidb")
            nc.sync.dma_start(out=cidb, in_=cidb_d.ap())
            tgtb = pp.tile([128, BL], F32, tag="tgtb")
            nc.sync.dma_start(out=tgtb, in_=tgtb_d.ap())
            negjb = pp.tile([128, BL], F32, tag="negjb")
            nc.sync.dma_start(out=negjb, in_=negjb_d.ap())

            # ---------- phase A: hardest-negative mining (exp domain) ----------
            eqm = []      # -onehot, kept for phase C target-logit extraction
            for t in range(NT):
                eq = pp.tile([128, C], F32, tag=f"eqm{t}")
                nc.vector.tensor_scalar(out=eq, in0=cidb,
                                        scalar1=tcols[:, t:t + 1], scalar2=None,
                                        op0=OP.is_equal)
                eqm.append(eq)
            negval = []
            for t in range(NT):
                dump = dp.tile([128, C], F32, tag="dump")
                rsum = smp.tile([128, 1], F32, tag="rsum")
                nc.scalar.activation(out=dump, in_=xloc[t], func=AF.Exp,
                                     bias=shA, scale=1.0, accum_out=rsum)
                negrr = smp.tile([128, 1], F32, tag="negrr")
                nc.vector.reciprocal(out=negrr, in_=rsum)
                nc.vector.tensor_scalar(out=negrr, in0=negrr, scalar1=-1.0,
                                        scalar2=None, op0=OP.mult)
                nv = nvp.tile([128, C], F32, tag="negval")
                nc.vector.scalar_tensor_tensor(out=nv, in0=dump, scalar=negrr,
                                               in1=eqm[t], op0=OP.mult,
                                               op1=OP.subtract)
                negval.append(nv)

            # transpose to [class, row]; top-8 over rows per class
            ccall = pp.tile([128, 4 * CT], F32, tag="ccall")
            vmaxcat = ccall[:, 0:CT]
            enccat = ccall[:, CT:2 * CT]
            for g in range(4):
                psts = []
                for ci in range(2):
                    pst = psa.tile([128, C], F32, tag=f"pst{ci}")
                    psts.append(pst)
                for t in range(NT):
                    for ci in range(2):
                        ct = g * 2 + ci
                        nc.tensor.transpose(
                            out=psts[ci][:, t * 128:(t + 1) * 128],
                            in_=negval[t][:, ct * 128:(ct + 1) * 128],
                            identity=ident)
                for ci in range(2):
                    ct = g * 2 + ci
                    nvT = sp.tile([128, C], F32, tag="nvT")
                    nc.scalar.copy(out=nvT, in_=psts[ci])
                    top8v = smp.tile([128, 8], F32, tag="top8v")
                    nc.vector.max(out=top8v, in_=nvT)
                    top8i = smp.tile([128, 8], U32, tag="top8i")
                    nc.vector.max_index(out=top8i, in_max=top8v, in_values=nvT)
                    idxf = smp.tile([128, 1], F32, tag="idxf")
                    nc.vector.tensor_copy(out=idxf, in_=top8i[:, 0:1])
                    nc.vector.tensor_copy(out=vmaxcat[:, ct:ct + 1],
                                          in_=top8v[:, 0:1])
                    # enc = (BIGI - core_off) - idx
                    nc.vector.tensor_scalar(out=enccat[:, ct:ct + 1], in0=idxf,
                                            scalar1=bigoff, scalar2=-1.0,
                                            op0=OP.subtract, op1=OP.mult)


            # ---------- phase B: first/second occurrence per class ----------
            f1cat = ccall[:, 2 * CT:3 * CT]
            f2cat = ccall[:, 3 * CT:4 * CT]
            eqB = []
            for ct in range(CT):
                eb = pp.tile([128, BL], F32, tag=f"eqB{ct}")
                nc.vector.tensor_tensor(
                    out=eb, in0=tgtb,
                    in1=cidcol[:, ct:ct + 1].to_broadcast([128, BL]),
                    op=OP.is_equal)
                eqB.append(eb)
                enb = sp.tile([128, BL], F32, tag="encB")
                nc.gpsimd.tensor_tensor(out=enb, in0=eb, in1=negjb, op=OP.mult)
                top8 = smp.tile([128, 8], F32, tag="top8b")
                nc.vector.max(out=top8, in_=enb)
                nc.vector.tensor_copy(out=f1cat[:, ct:ct + 1], in_=top8[:, 0:1])
                nc.vector.tensor_copy(out=f2cat[:, ct:ct + 1], in_=top8[:, 1:2])
            nc.sync.dma_start(out=cc_in.ap(), in_=ccall)

            # ---------- AllGather + global combine ----------
            nc.gpsimd.collective_compute(
                "AllGather", OP.bypass,
                replica_groups=[list(range(NCORES))],
                ins=[cc_in.ap().opt()], outs=[cc_out.ap().opt()])

            g8 = pp.tile([128, NCORES, 4 * CT + 1], F32, tag="g8")
            gsrc = bass.AP(tensor=cc_out.ap().tensor, offset=0,
                           ap=[[4 * CT, 128], [128 * 4 * CT, NCORES],
                               [1, 4 * CT]])
            nc.scalar.dma_start(out=g8[:, :, 0:4 * CT], in_=gsrc)

            def qslice(q, ct):
                return g8[:, 0:NCORES, q * CT + ct]

            rhs = []
            for ct in range(CT):
                # hardest negative: max value across cores, tie -> max enc
                gv = smp.tile([128, 1], F32, tag="gv")
                nc.vector.tensor_reduce(out=gv, in_=qslice(0, ct), axis=AX.X,
                                        op=OP.max)
                mm = smp.tile([128, NCORES], F32, tag="mm")
                nc.vector.tensor_tensor(out=mm, in0=qslice(0, ct),
                                        in1=gv.to_broadcast([128, NCORES]),
                                        op=OP.is_ge)
                cand = smp.tile([128, NCORES], F32, tag="cand")
                nc.vector.tensor_tensor(out=cand, in0=mm, in1=qslice(1, ct),
                                        op=OP.mult)
                genc = smp.tile([128, 1], F32, tag="genc")
                nc.vector.tensor_reduce(out=genc, in_=cand, axis=AX.X, op=OP.max)

                rt = pp.tile([128, 3], F32, tag=f"rhs{ct}")
                nc.vector.tensor_scalar(out=rt[:, 2:3], in0=genc, scalar1=-1.0,
                                        scalar2=BIGI, op0=OP.mult, op1=OP.add)
                # pos: two smallest global indices of this class
                cat = smp.tile([128, 2 * NCORES], F32, tag="cat")
                nc.vector.tensor_copy(out=cat[:, 0:NCORES], in_=qslice(2, ct))
                nc.vector.tensor_copy(out=cat[:, NCORES:], in_=qslice(3, ct))
                topg = smp.tile([128, 8], F32, tag="topg")
                nc.vector.max(out=topg, in_=cat)
                nc.vector.tensor_scalar(out=rt[:, 0:1], in0=topg[:, 0:1],
                                        scalar1=-1.0, scalar2=BIGI,
                                        op0=OP.mult, op1=OP.add)
                nc.vector.tensor_scalar(out=rt[:, 1:2], in0=topg[:, 1:2],
                                        scalar1=-1.0, scalar2=BIGI,
                                        op0=OP.mult, op1=OP.add)
                rhs.append(rt)

            # ---------- per-anchor resolution + paired gathers ----------
            off2 = pp.tile([128, 2 * NT], I32, tag="off2")
            xpn_t = []
            for t in range(NT):
                ps = psb.tile([128, 3], F32, tag="ps3")
                for ct in range(CT):
                    nc.tensor.matmul(ps, lhsT=eqB[ct][:, t * 128:(t + 1) * 128],
                                     rhs=rhs[ct], start=(ct == 0),
                                     stop=(ct == CT - 1))
                # pos = (g1 == i) ? (g2 absent ? 0 : g2) : g1
                m1 = smp.tile([128, 1], F32, tag="m1")
                nc.vector.tensor_tensor(out=m1, in0=ps[:, 0:1],
                                        in1=gidxcol[:, t:t + 1], op=OP.is_equal)
                m2n = smp.tile([128, 1], F32, tag="m2n")
                nc.vector.tensor_scalar(out=m2n, in0=ps[:, 1:2], scalar1=BIGI,
                                        scalar2=None, op0=OP.is_lt)
                p2z = smp.tile([128, 1], F32, tag="p2z")
                nc.vector.tensor_tensor(out=p2z, in0=ps[:, 1:2], in1=m2n,
                                        op=OP.mult)
                d = smp.tile([128, 1], F32, tag="dsel")
                nc.vector.tensor_tensor(out=d, in0=p2z, in1=ps[:, 0:1],
                                        op=OP.subtract)
                nc.vector.tensor_tensor(out=d, in0=m1, in1=d, op=OP.mult)
                posf = smp.tile([128, 1], F32, tag="posf")
                nc.vector.tensor_tensor(out=posf, in0=ps[:, 0:1], in1=d,
                                        op=OP.add)
                nc.vector.tensor_copy(out=off2[:, 2 * t:2 * t + 1], in_=posf)
                nc.vector.tensor_copy(out=off2[:, 2 * t + 1:2 * t + 2],
                                      in_=ps[:, 2:3])
                xp = gp.tile([128, C], F32, tag="xpn")
                xn = gp.tile([128, C], F32, tag="xng")
                csplit = 2 if t == 0 else 1
                for tl_, col in ((xp, 2 * t), (xn, 2 * t + 1)):
                    for s in range(csplit):
                        lo, hi = s * C // csplit, (s + 1) * C // csplit
                        nc.gpsimd.indirect_dma_start(
                            out=tl_[:, lo:hi], out_offset=None,
                            in_=x_d.ap(), element_offset=lo,
                            in_offset=bass.IndirectOffsetOnAxis(
                                ap=off2[:, col:col + 1], axis=0))
                xpn_t.append((xp, xn))

            # ---------- phase C: CE over summed logits ----------
            for t in range(NT):
                nc.vector.tensor_tensor(out=xloc[t], in0=xloc[t],
                                        in1=xpn_t[t][0], op=OP.add)
                nc.vector.tensor_tensor(out=xloc[t], in0=xloc[t],
                                        in1=xpn_t[t][1], op=OP.add)

            rsums3 = []
            exps3 = []
            for t in range(NT):
                dump = dp.tile([128, C], F32, tag="dump")
                rsum = smp.tile([128, 1], F32, tag=f"rsumC{t}")
                e = nc.scalar.activation(out=dump, in_=xloc[t], func=AF.Exp,
                                         bias=shC, scale=1.0, accum_out=rsum)
                exps3.append(e)
                rsums3.append(rsum)
            lnrs3 = []
            for t in range(NT):
                lnr = smp.tile([128, 1], F32, tag=f"lnrC{t}")
                ln = nc.scalar.activation(out=lnr, in_=rsums3[t], func=AF.Ln)
                # keep ACT on one table at a time: all Exps, then all Lns
                add_dep_helper(ln.ins, exps3[-1].ins, sync=False)
                lnrs3.append(lnr)

            acc = pp.tile([128, 1], F32, tag="acc")
            for t in range(NT):
                prod = sp.tile([128, C], F32, tag="nvT")
                nc.vector.tensor_tensor(out=prod, in0=xloc[t], in1=eqm[t],
                                        op=OP.mult)
                tvr = smp.tile([128, 1], F32, tag="tv")
                nc.vector.tensor_reduce(out=tvr, in_=prod, axis=AX.X, op=OP.add)
                # li = lse - tval = (ln(rsum) + SHIFT_C) + tvr   (tvr = -tval)
                li = smp.tile([128, 1], F32, tag="li")
                nc.vector.tensor_scalar(out=li, in0=lnrs3[t], scalar1=SHIFT_C,
                                        scalar2=None, op0=OP.add)
                nc.vector.tensor_tensor(out=li, in0=li, in1=tvr, op=OP.subtract)
                if t == 0:
                    nc.vector.tensor_copy(out=acc, in_=li)
                else:
                    nc.vector.tensor_tensor(out=acc, in0=acc, in1=li, op=OP.add)

            # partition-sum via PE: acc.T @ ones = [1,1]
            pss = psb.tile([1, 1], F32, tag="psum_out")
            nc.tensor.matmul(pss, lhsT=acc, rhs=ones, start=True, stop=True)
            outt = smp.tile([1, 1], F32, tag="outt")
            nc.vector.tensor_copy(out=outt, in_=pss)
            nc.sync.dma_start(out=out_d.ap(), in_=outt)

    nc.compile()
    return nc


_NC_CACHE = {}


def get_nc():
    if "nc" not in _NC_CACHE:
        _NC_CACHE["nc"] = build_nc()
    return _NC_CACHE["nc"]


def make_in_maps(x, target):
    x = np.ascontiguousarray(np.asarray(x, dtype=np.float32))
    tgt = np.asarray(target).astype(np.int64)
    assert x.shape == (B, C) and tgt.shape == (B,)

    cid = np.arange(C, dtype=np.float32)
    cidb_full = np.ascontiguousarray(np.broadcast_to(cid, (128, C)))
    ident_full = np.eye(128, dtype=np.float32)
    cidcol = np.ascontiguousarray(cid.reshape(CT, 128).T)

    in_maps = []
    for k in range(NCORES):
        rows = slice(k * BL, (k + 1) * BL)
        tl = tgt[rows].astype(np.float32)
        gi = (k * BL + np.arange(BL)).astype(np.float32)
        nj = BIGI - gi
        in_maps.append({
            "x": x,
            "xloc": np.ascontiguousarray(x[rows]),
            "cidb": cidb_full,
            "tgtb": np.ascontiguousarray(np.broadcast_to(tl, (128, BL))),
            "negjb": np.ascontiguousarray(np.broadcast_to(nj, (128, BL))),
            "ident": ident_full,
            "tcols": np.ascontiguousarray(tl.reshape(NT, 128).T),
            "gidxcol": np.ascontiguousarray(gi.reshape(NT, 128).T),
            "cidcol": cidcol,
            "bigoff": np.full((128, 1), BIGI - k * BL, dtype=np.float32),
        })
    return in_maps


def kernel(x, target):
    nc = get_nc()
    in_maps = make_in_maps(x, target)
    res = run_bass_kernel_spmd(nc, in_maps, core_ids=list(range(NCORES)))
    total = sum(float(res.results[k]["partial"][0, 0]) for k in range(NCORES))
    return np.float32(total / B)

